# revision 17
# baseline (speedup 1.0000x reference)
"""DGI (Deep Graph Infomax) forward on 8 TRN2 NeuronCores.

Strategy (dst-sharded, host-pregathered fp8 message stream, no collective):
  - Nodes split into 8 contiguous dst ranges of 6250; core k owns all edges
    whose destination lands in its range, so the scatter-add is fully local.
  - Math identity: gcn(x) = ((A+I) @ (dinv*x)) * dinv_dst @ W + b.
    Aggregation commutes with W, so we aggregate RAW features (both branches
    concatenated: 256 feats/edge) and apply W once on the sharded result.
  - The per-edge message stream is float8_e3m4 (scaled by S=8; 1/S is folded
    into the W passed to the device).  Host-side ERROR-FEEDBACK quantization
    propagates each edge's rounding residual into the next edge of the same
    dst node, so residuals cancel in the aggregation sum (order-independent)
    -- end-to-end rel err ~6e-3 vs 2.9e-2 for naive fp8 RNE.
  - Scatter-add on the TensorEngine per dst WINDOW of 64: each 128-edge chunk
    contributes matmul(msgs[128e,128f], S[128e,64d]) accumulated in PSUM.
    Windows of 64 (vs 128) halve the matmul streaming time and quarter the
    DVE one-hot volume.  One-hot S built on-chip by DVE is_equal from a tiny
    [128, nchk] dst-local table.
  - BatchNorm is training-mode batch stats over ALL nodes.  Instead of an
    AllReduce (measured ~130us stall: mesh-protocol latency + inter-core
    skew), each core outputs its partial (sum, sumsq) stats [128,2] and its
    h1 shard [128,6250] fp16; the HOST does the trivial 2-level reduction and
    the final matvec  sc1 = h1 @ (rstd*gamma*wc) + const  (0.4% of FLOPs).
  - Branch 2 needs no BN: sc2 = h2 @ wc + disc_b computed fully on device,
    with wc = disc_W @ sigmoid(beta) host-precomputed (mean of BN output is
    exactly beta, so c = sigmoid(beta)).
  - Per-tile pipeline (13 PSUM tiles of 8 windows): DMA msgs -> DVE one-hot
    -> PE scatter -> PSUM->SBUF copy, with branch post-processing (W matmul,
    PReLU, BN partial stats, sc2 matvec, h1 DMA-out) interleaved one tile
    behind.
"""

import numpy as np
import ml_dtypes

N = 50000
FB = 128                    # features per branch
F = 256                     # concat features (both branches)
N_CORES = 8
NPC = N // N_CORES          # 6250 nodes per core
CHK = 128                   # edges per chunk (PE partition dim)
WIN = 64                    # dst window width (one-hot free dim)
NWIN = (NPC + WIN - 1) // WIN               # 98 windows per core
NPAD = NWIN * WIN                           # 6272 dst slots per core
TILE_WINS = 8                               # windows per PSUM tile (512 cols)
NT = (NWIN + TILE_WINS - 1) // TILE_WINS    # 13 PSUM tiles
EPS = 1e-5
S_SCALE = 8.0               # fp8 pre-scale; 1/S folded into W on host
F8MAX = 15.5                # e3m4 max normal

_cache = {}


def _preprocess(x, x_permute, edge_index):
    """Host: degree/norm, stream positions, error-feedback fp8 quantization."""
    src = np.concatenate([np.asarray(edge_index[0], np.int64), np.arange(N)])
    dst = np.concatenate([np.asarray(edge_index[1], np.int64), np.arange(N)])
    T = len(src)
    deg = np.bincount(dst, minlength=N).astype(np.float32)  # >=1 (self loops)
    dinv = (1.0 / np.sqrt(deg)).astype(np.float32)

    xg = np.concatenate([x, x_permute], axis=1) * dinv[:, None]  # [N,256] f32

    # ---- stream slot assignment: sort by (core, window), chunk by 128 ----
    core = dst // NPC                        # [T]
    wloc = (dst % NPC) // WIN                # [T] 0..NWIN-1
    key = core * NWIN + wloc
    order = np.argsort(key, kind="stable")
    key_s = key[order]

    counts = np.bincount(key, minlength=N_CORES * NWIN).reshape(N_CORES, NWIN)
    # uniform #chunks per window across cores (SPMD: same program)
    ncall = np.maximum((counts.max(axis=0) + CHK - 1) // CHK, 1)   # [NWIN]
    nchk = int(ncall.sum())
    woff = np.zeros(NWIN, np.int64)
    woff[1:] = np.cumsum(ncall)[:-1]

    starts = np.zeros(N_CORES * NWIN + 1, np.int64)
    starts[1:] = np.cumsum(counts.reshape(-1))
    rank_g = np.arange(T) - starts[key_s]
    pos = woff[key_s % NWIN] * CHK + rank_g
    # per-edge slot coords in ORIGINAL edge order
    e_core = np.empty(T, np.int32)
    e_row = np.empty(T, np.int32)
    e_chk = np.empty(T, np.int32)
    e_core[order] = (key_s // NWIN).astype(np.int32)
    e_row[order] = (pos % CHK).astype(np.int32)
    e_chk[order] = (pos // CHK).astype(np.int32)

    # ---- error-feedback quantization, per-dst carry chains ----
    dorder = np.argsort(dst, kind="stable")
    dst_d = dst[dorder]
    dstarts = np.searchsorted(dst_d, np.arange(N))
    drank = np.arange(T) - dstarts[dst_d]
    maxdeg = int(drank.max()) + 1

    xm = np.zeros((N_CORES, CHK, nchk, F), ml_dtypes.float8_e3m4)
    carry = np.zeros((N, F), np.float32)
    nrm = dinv * np.float32(S_SCALE)
    for r in range(maxdeg):
        sel = dorder[drank == r]             # edge ids, unique dst within rank
        d = dst[sel]
        v = xg[src[sel]] * nrm[d][:, None] + carry[d]
        q = np.clip(v, -F8MAX, F8MAX).astype(ml_dtypes.float8_e3m4)
        carry[d] = v - q.astype(np.float32)
        xm[e_core[sel], e_row[sel], e_chk[sel], :] = q

    dl = np.zeros((N_CORES, CHK, nchk), np.float16)
    dl[e_core, e_row, e_chk] = ((dst % NPC) % WIN).astype(np.float16)
    io = np.tile(np.arange(WIN, dtype=np.float16), (CHK, 1))

    return xm.reshape(N_CORES, CHK, nchk * F), dl, io, ncall, woff, nchk


def _build_program(ncall, woff, nchk):
    import concourse.bacc as bacc
    import concourse.mybir as mybir
    import concourse.tile as tile

    nc = bacc.Bacc("TRN2", target_bir_lowering=False, debug=False,
                   enable_asserts=False, num_devices=N_CORES)
    dt = mybir.dt
    AF = mybir.ActivationFunctionType
    ALU = mybir.AluOpType

    xm_d = nc.dram_tensor("xm", [CHK, nchk * F], dt.float8e3,
                          kind="ExternalInput")
    dl_d = nc.dram_tensor("dl", [CHK, nchk], dt.float16, kind="ExternalInput")
    io_d = nc.dram_tensor("io", [CHK, WIN], dt.float16, kind="ExternalInput")
    w_d = nc.dram_tensor("w", [FB, FB], dt.float16, kind="ExternalInput")
    wc_d = nc.dram_tensor("wcv", [FB, 1], dt.float16, kind="ExternalInput")
    # small vectors: [128, 2] = (b, prelu_a)
    sv_d = nc.dram_tensor("sv", [FB, 2], dt.float32, kind="ExternalInput")
    # small scalars: [1, 1] = (s2,)
    sc_d = nc.dram_tensor("sc", [1, 1], dt.float32, kind="ExternalInput")
    out2_d = nc.dram_tensor("out2", [1, NPC], dt.float32,
                            kind="ExternalOutput")
    h1_d = nc.dram_tensor("h1o", [FB, NPC], dt.float16, kind="ExternalOutput")
    st_d = nc.dram_tensor("st", [FB, 2], dt.float32, kind="ExternalOutput")

    # per-PSUM-tile metadata
    tiles = []
    for t in range(NT):
        w0 = t * TILE_WINS
        wend = min(w0 + TILE_WINS, NWIN)
        toff = int(woff[w0])
        tend = int(woff[wend - 1] + ncall[wend - 1])
        tiles.append((w0, wend, toff, tend - toff))

    with tile.TileContext(nc) as tc:
        with tc.tile_pool(name="mt", bufs=4) as mt_p, \
             tc.tile_pool(name="smat", bufs=3) as smat_p, \
             tc.tile_pool(name="big", bufs=1) as big_p, \
             tc.tile_pool(name="small", bufs=1) as small_p, \
             tc.tile_pool(name="scr", bufs=3) as scr_p, \
             tc.tile_pool(name="ps1", bufs=2, space="PSUM") as ps1_p, \
             tc.tile_pool(name="ps2", bufs=2, space="PSUM") as ps2_p, \
             tc.tile_pool(name="ph2", bufs=2, space="PSUM") as ph2_p:

            agg1 = big_p.tile([FB, NPAD], dt.float16)   # branch-1 agg^T
            agg2 = big_p.tile([FB, NPAD], dt.float16)
            h1 = big_p.tile([FB, NPAD], dt.float16)     # prelu(agg1@W+b)^T
            w_t = small_p.tile([FB, FB], dt.float16)
            wc16 = small_p.tile([FB, 1], dt.float16)
            sv = small_p.tile([FB, 2], dt.float32)
            scs = small_p.tile([1, 1], dt.float32)
            dl_t = small_p.tile([CHK, nchk], dt.float16)
            io_t = small_p.tile([CHK, WIN], dt.float16)
            sums = small_p.tile([FB, NT], dt.float32)
            sumsq = small_p.tile([FB, NT], dt.float32)
            st2 = small_p.tile([FB, 2], dt.float32)
            out2 = small_p.tile([1, NPC], dt.float32)
            b_ap, a_ap = sv[:, 0:1], sv[:, 1:2]

            # small constants first (~1us total) so the one-hot/compute
            # pipeline isn't head-blocked behind the first 2.4MB stream DMA
            nc.sync.dma_start(dl_t[:], dl_d[:])
            nc.sync.dma_start(io_t[:], io_d[:])
            nc.sync.dma_start(w_t[:], w_d[:])
            nc.sync.dma_start(wc16[:], wc_d[:])
            nc.sync.dma_start(sv[:], sv_d[:])
            nc.sync.dma_start(scs[:], sc_d[:])

            mts = [None] * NT
            def issue_mt(t):
                _, _, toff, csz = tiles[t]
                mts[t] = mt_p.tile([CHK, csz * F], dt.float8e3, tag="mt",
                                   name="mt")
                nc.sync.dma_start(mts[t][:], xm_d[:, toff * F:(toff + csz) * F])

            issue_mt(0)
            issue_mt(1)
            issue_mt(2)

            # HAM warm-up: ~8us of throwaway matmuls while the first stream
            # DMA is in flight, so real scatter matmuls start at 2.4 GHz
            wps = ph2_p.tile([CHK, 512], dt.float32, tag="ph2")
            for _ in range(120):
                nc.tensor.matmul(wps[0:WIN, 0:WIN], io_t[:], io_t[:],
                                 start=True, stop=True)

            # phase-2 post-processing is software-pipelined so no PE
            # instruction ever waits on same-iteration ACT work:
            #   stage MM (t-1 behind scatter): W^T@agg matmuls for both branches
            #   stage ACT (same lag):          prelu/stats/h1-out/h2
            #   stage TAIL (t-2 behind):       sc2 matvec + identity
            ps_a = [None] * NT
            ps_b = [None] * NT
            h2s = [None] * NT
            ps_ss = [None] * NT

            def p2_width(t):
                return min(512, NPC - t * 512)

            def phase2_mms(t):
                c0, w = t * 512, p2_width(t)
                ps_a[t] = ph2_p.tile([FB, 512], dt.float32, tag="ph2",
                                     name="psa")
                nc.tensor.matmul(ps_a[t][:, :w], w_t[:], agg1[:, c0:c0 + w],
                                 start=True, stop=True)
                ps_b[t] = ph2_p.tile([FB, 512], dt.float32, tag="ph2",
                                     name="psb")
                nc.tensor.matmul(ps_b[t][:, :w], w_t[:], agg2[:, c0:c0 + w],
                                 start=True, stop=True)

            def phase2_act(t):
                c0, w = t * 512, p2_width(t)
                nc.scalar.activation(h1[:, c0:c0 + w], ps_a[t][:, :w],
                                     AF.Prelu, bias=b_ap, alpha=a_ap,
                                     accum_out=sums[:, t:t + 1])
                sq = scr_p.tile([FB, 512], dt.float32, tag="sq")
                nc.scalar.activation(sq[:, :w], h1[:, c0:c0 + w], AF.Square,
                                     accum_out=sumsq[:, t:t + 1])
                nc.scalar.dma_start(h1_d[:, c0:c0 + w], h1[:, c0:c0 + w])
                h2s[t] = scr_p.tile([FB, 512], dt.float16, tag="h2",
                                    name="h2")
                nc.scalar.activation(h2s[t][:, :w], ps_b[t][:, :w], AF.Prelu,
                                     bias=b_ap, alpha=a_ap)

            def phase2_matvec(t):
                w = p2_width(t)
                ps_ss[t] = ph2_p.tile([1, 512], dt.float32, tag="ps_s",
                                      name="ps_s")
                nc.tensor.matmul(ps_ss[t][:, :w], wc16[:], h2s[t][:, :w],
                                 start=True, stop=True)

            def phase2_ident(t):
                c0, w = t * 512, p2_width(t)
                nc.scalar.activation(out2[:, c0:c0 + w], ps_ss[t][:, :w],
                                     AF.Identity, bias=scs[0:1, 0:1])

            # ---- main loop: stream msgs + DVE one-hot + PE window scatter,
            #      previous tile's post-processing interleaved ----
            for t in range(NT):
                w0, wend, toff, csz = tiles[t]
                tw = (wend - w0) * WIN
                mt = mts[t]
                s3 = smat_p.tile([CHK, csz, WIN], dt.float8e3, tag="s3")
                nc.vector.tensor_tensor(
                    s3[:],
                    io_t[:].unsqueeze(1).broadcast_to((CHK, csz, WIN)),
                    dl_t[:, toff:toff + csz].unsqueeze(2).broadcast_to(
                        (CHK, csz, WIN)),
                    op=ALU.is_equal)
                ps_lo = ps1_p.tile([FB, 512], dt.float32, tag="ps_lo")
                ps_hi = ps2_p.tile([FB, 512], dt.float32, tag="ps_hi")
                for w in range(w0, wend):
                    base = int(woff[w]) - toff
                    nb = int(ncall[w])
                    o0 = (w - w0) * WIN
                    for j in range(nb):
                        cj = base + j
                        nc.tensor.matmul(ps_lo[:, o0:o0 + WIN],
                                         mt[:, cj * F:cj * F + FB],
                                         s3[:, cj, :],
                                         start=(j == 0), stop=(j == nb - 1))
                        nc.tensor.matmul(ps_hi[:, o0:o0 + WIN],
                                         mt[:, cj * F + FB:(cj + 1) * F],
                                         s3[:, cj, :],
                                         start=(j == 0), stop=(j == nb - 1))
                # phase-2 stages lag the scatter by 2-3 tiles so their
                # dependencies are long-satisfied when the FIFO PE queue
                # reaches them -- no head-of-line blocking of the next
                # scatter burst
                if t >= 2:
                    phase2_mms(t - 2)
                if t >= 3:
                    phase2_matvec(t - 3)
                if t + 3 < NT:
                    issue_mt(t + 3)
                if t >= 2:
                    phase2_act(t - 2)
                if t >= 3:
                    phase2_ident(t - 3)
                nc.scalar.copy(out=agg1[:, t * 512:t * 512 + tw],
                               in_=ps_lo[:, :tw])
                nc.scalar.copy(out=agg2[:, t * 512:t * 512 + tw],
                               in_=ps_hi[:, :tw])

            phase2_mms(NT - 2)
            phase2_matvec(NT - 3)
            phase2_act(NT - 2)
            phase2_ident(NT - 3)
            phase2_mms(NT - 1)
            phase2_matvec(NT - 2)
            phase2_act(NT - 1)
            phase2_ident(NT - 2)
            phase2_matvec(NT - 1)
            phase2_ident(NT - 1)

            # ---- stats out ----
            nc.vector.tensor_reduce(st2[:, 0:1], sums[:],
                                    mybir.AxisListType.X, ALU.add)
            nc.vector.tensor_reduce(st2[:, 1:2], sumsq[:],
                                    mybir.AxisListType.X, ALU.add)
            nc.scalar.dma_start(st_d[:], st2[:])
            nc.scalar.dma_start(out2_d[:], out2[:])

    nc.compile()
    return nc


def kernel(x, x_permute, edge_index, W, b, prelu_a, bn_gamma, bn_beta,
           disc_W, disc_b):
    from concourse.bass_utils import run_bass_kernel_spmd

    x = np.asarray(x, np.float32)
    x_permute = np.asarray(x_permute, np.float32)
    xm, dl, io, ncall, woff, nchk = _preprocess(x, x_permute, edge_index)

    key = (tuple(ncall.reshape(-1)), nchk)
    if key not in _cache:
        _cache[key] = _build_program(ncall, woff, nchk)
    nc = _cache[key]

    W = np.asarray(W, np.float32)
    bv = np.asarray(b, np.float32)
    gamma = np.asarray(bn_gamma, np.float32)
    beta = np.asarray(bn_beta, np.float32)
    disc_W = np.asarray(disc_W, np.float32)
    a = float(np.asarray(prelu_a))
    db = float(np.asarray(disc_b))
    c = 1.0 / (1.0 + np.exp(-beta.astype(np.float64)))
    wc = (disc_W.astype(np.float64) @ c).astype(np.float32)
    sv = np.stack([bv, np.full(FB, a, np.float32)], axis=1)
    sc = np.array([[db]], np.float32)
    w_dev = (W / np.float32(S_SCALE)).astype(np.float16)  # undo fp8 pre-scale
    wc16 = wc.astype(np.float16).reshape(FB, 1)

    in_maps = [{"xm": xm[cid], "dl": dl[cid], "io": io, "w": w_dev,
                "wcv": wc16, "sv": sv, "sc": sc} for cid in range(N_CORES)]
    res = run_bass_kernel_spmd(nc, in_maps, core_ids=list(range(N_CORES)))

    # ---- host finish: 2-level BN stats + final matvec (0.4% of FLOPs) ----
    sums = np.zeros(FB, np.float64)
    sumsq = np.zeros(FB, np.float64)
    for cid in range(N_CORES):
        st = res.results[cid]["st"].astype(np.float64)
        sums += st[:, 0]
        sumsq += st[:, 1]
    mu = (sums / N).astype(np.float32)
    var = (sumsq / N - (sums / N) ** 2).astype(np.float32)
    rstd = 1.0 / np.sqrt(var + np.float32(EPS))
    wc1 = rstd * gamma * wc
    const1 = np.float32(db + float(beta.astype(np.float64) @ wc.astype(np.float64))
                        - float(mu.astype(np.float64) @ wc1.astype(np.float64)))

    out = np.empty(2 * N, np.float32)
    for cid in range(N_CORES):
        h1c = res.results[cid]["h1o"].astype(np.float32)   # [128, NPC]
        out[cid * NPC:(cid + 1) * NPC] = wc1 @ h1c + const1
        out[N + cid * NPC:N + (cid + 1) * NPC] = res.results[cid]["out2"][0]
    return out
